# revision 34
# baseline (speedup 1.0000x reference)
"""Trainium2 Bass kernel for the AudioVisualModel contrastive loss.

Problem (hardcoded shapes): B=32, Na=50, Nv=1024, D=1024, fp32 inputs.

reference:
    a = l2norm(audio)  (B,Na,D); v = l2norm(visual) (B,Nv,D)
    token_sims = einsum('iad,jvd->ijav', a, v) / T
    clip_sims  = mean_a max_v token_sims          (B,B)
    loss = InfoNCE(clip_sims/sqrt(B)) + 0.01*mean(min(token_sims,0)^2)
           + 0.1*max(-log T,0)^2

Sharding: core c owns audio clips [4c, 4c+4) (its 4 rows of clip_sims);
every core holds the full visual tensor (replicated), passed transposed as
(j, d, v) so the contraction dim d lands on SBUF partitions.  Each core
outputs 4x32 row sums (sum over audio tokens of max_v of the UNtempered
cosine sims) plus its partial sum of min(cos,0)^2; the host divides by
T / T^2 (T>0 so max commutes), assembles the 32x32 matrix and does the
tiny symmetric log-softmax + regularizer combine.

Raw bass (no Tile): this image's walrus rejects Tile's multi-wait
instructions and the bass_isa custom GPSIMD ops, so the kernel is five
hand-synchronized engine streams:
  GPSIMD: identity build + one 4MB fp32->bf16 cast-DMA per visual clip
  PE:     per (j, v-block): 8 accumulating bf16 matmuls for sims(v,a)
          and 8 for diag(V^T V) norms; audio transposes at startup
  DVE:    diag extract (mult by I + reduce), reciprocal, fused
          scale+running-max (scalar_tensor_tensor), min(x,0) capture,
          partition-max tree, per-clip segment sums
  ACT:    sqrt (rsqrt = DVE recip + ACT sqrt), Square+accumulate for the
          nonneg regularizer
  SYNC:   audio loads, output stores
"""

import numpy as np

B, NA, NV, D = 32, 50, 1024, 1024
N_CORES = 8
CPC = B // N_CORES          # clips per core = 4
ATOK = CPC * NA             # audio tokens per core = 200
AGRP = ATOK // 2            # audio tokens per partition group = 100
DCH = D // 128              # contraction chunks = 8
VBLK = NV // 128            # v-blocks per clip = 8
G = B * VBLK                # 256 global v-block iterations

_CACHED_NC = None
TRACE = False
LAST_EXEC_NS = None
LAST_TRACE_PATH = None


def _build_bass():
    from contextlib import ExitStack

    import concourse.bass as bass
    import concourse.mybir as mybir

    f32 = mybir.dt.float32
    bf16 = mybir.dt.bfloat16
    Alu = mybir.AluOpType
    Act = mybir.ActivationFunctionType
    X = mybir.AxisListType.X

    nc = bass.Bass()
    a_in = nc.declare_dram_parameter("audio", [ATOK, D], f32, isOutput=False)
    vt_in = nc.declare_dram_parameter("vt", [B, D, NV], f32, isOutput=False)
    # per-audio-token maxima, 7 swizzled 32-col blocks per visual clip j:
    # out[i, j*7+b] = max_v sims[v, a=32b+i]   (a >= 200 are padding)
    cs_out = nc.declare_dram_parameter(
        "clip_rows", [32, 7 * B], f32, isOutput=True
    )
    nn_out = nc.declare_dram_parameter("nonneg", [128, 1], f32, isOutput=True)

    ctx = ExitStack()
    with ctx:
        # ---- SBUF ----
        sb = lambda name, shape, dt: ctx.enter_context(
            nc.sbuf_tensor(name, shape, dt)
        )
        ident = sb("ident", [128, 128], bf16)
        aT = sb("aT", [128, DCH, ATOK], bf16)
        a_nat = [sb(f"a_nat{i}", [AGRP, D], f32) for i in range(2)]
        sqtmp = sb("sqtmp", [AGRP, D], f32)
        rs_a = [sb(f"rs_a{i}", [AGRP, 1], f32) for i in range(2)]
        a_bf = [sb(f"a_bf{i}", [AGRP, D], bf16) for i in range(2)]
        vt = [sb(f"vt{i}", [128, DCH, NV], bf16) for i in range(2)]
        dtmp = [sb(f"dtmp{i}", [128, 128], bf16) for i in range(2)]
        ssq = sb("ssq", [128, 4], f32)
        rs = sb("rs", [128, 4], f32)
        neg = sb("neg", [128, 4, ATOK], bf16)
        sqjunk = sb("sqjunk", [128, ATOK], bf16)
        runmax = sb("runmax", [128, 224], f32)   # 200 real a-cols + 24 pad
        trout = sb("trout", [32, 224], f32)
        bandtmp = sb("bandtmp", [32, 7], f32)
        maxtok = sb("maxtok", [32, 7 * B], f32)
        nn_buf = sb("nn_buf", [128, G], f32)
        nn_col = sb("nn_col", [128, 1], f32)

        # ---- PSUM ----  (8 banks: 3 norm + 3 sims + 2 audio-transpose)
        ps_n = [
            ctx.enter_context(nc.psum_tensor(f"ps_n{i}", [128, 128], f32))
            for i in range(3)
        ]
        ps_s = [
            ctx.enter_context(nc.psum_tensor(f"ps_s{i}", [128, ATOK], f32))
            for i in range(3)
        ]
        apsum = [
            ctx.enter_context(nc.psum_tensor(f"apsum{i}", [128, AGRP], bf16))
            for i in range(2)
        ]

        # ---- semaphores ----
        sems = {}
        for name in [
            "IDZ", "ID", "ADMA", "DVA", "ACTA", "DVB", "PET", "ATR",
            "VT0", "VT1", "VTFREE", "PEVB", "PSFREE", "RECIP", "SQRT", "ACTSQ",
            "DVEDONE", "OUTD",
        ]:
            sems[name] = ctx.enter_context(nc.semaphore(name))

        with nc.Block() as block:

            @block.sync
            def _(sync):
                sync.dma_start(out=a_nat[0][:], in_=a_in[0:AGRP, :]).then_inc(
                    sems["ADMA"], 16
                )
                sync.dma_start(
                    out=a_nat[1][:], in_=a_in[AGRP:ATOK, :]
                ).then_inc(sems["ADMA"], 16)
                # outputs at the very end
                sync.wait_ge(sems["DVEDONE"], B + 1)
                sync.dma_start(out=cs_out[:, :], in_=maxtok[:]).then_inc(
                    sems["OUTD"], 16
                )
                sync.dma_start(
                    out=nn_out[:, :], in_=nn_col[:]
                ).then_inc(sems["OUTD"], 16)
                sync.wait_ge(sems["OUTD"], 32)

            @block.gpsimd
            def _(gpsimd):
                gpsimd.wait_ge(sems["IDZ"], 1)
                gpsimd.affine_select(
                    out=ident[:], in_=ident[:],
                    compare_op=Alu.not_equal, fill=1.0,
                    base=0, pattern=[[-1, 128]], channel_multiplier=1,
                ).then_inc(sems["ID"], 1)
                for j in range(B):
                    if j >= 2:
                        gpsimd.wait_ge(sems["VTFREE"], j - 1)
                    gpsimd.dma_start(
                        out=vt[j % 2][:],
                        in_=vt_in[j].rearrange("(c p) v -> p c v", p=128),
                    ).then_inc(sems[f"VT{j % 2}"], 16)

            @block.tensor
            def _(tensor):
                tensor.wait_ge(sems["ID"], 1)
                # audio transposes: a_bf (100, 1024) -> aT (128, ch, grp*100)
                for grp in range(2):
                    tensor.wait_ge(sems["DVB"], grp + 1)
                    for ch in range(DCH):
                        t = grp * DCH + ch
                        if t >= 2:
                            tensor.wait_ge(sems["ATR"], t - 1)
                        nc.tensor.transpose(
                            apsum[t % 2][:],
                            a_bf[grp][:, ch * 128 : (ch + 1) * 128],
                            ident[:AGRP, :AGRP],
                        ).then_inc(sems["PET"], 1)
                # main loop
                for g in range(G):
                    j, vb = divmod(g, VBLK)
                    if vb == 0:
                        tensor.wait_ge(sems[f"VT{j % 2}"], 16 * (j // 2 + 1))
                        if g == 0:
                            tensor.wait_ge(sems["ATR"], 16)
                    if g >= 3:
                        tensor.wait_ge(sems["PSFREE"], g - 2)
                    for ch in range(DCH):
                        w = vt[j % 2][:, ch, vb * 128 : (vb + 1) * 128]
                        mm = nc.tensor.matmul(
                            ps_n[g % 3][:], w, w,
                            start=(ch == 0), stop=(ch == DCH - 1),
                        )
                        if vb == 0 and ch == 0 and j >= 1:
                            mm.then_inc(sems["VTFREE"], 1)
                        mm2 = nc.tensor.matmul(
                            ps_s[g % 3][:], w, aT[:, ch, :],
                            start=(ch == 0), stop=(ch == DCH - 1),
                        )
                        if ch == DCH - 1:
                            mm2.then_inc(sems["PEVB"], 1)

            @block.vector
            def _(vector):
                vector.memset(ident[:], 0.0).then_inc(sems["IDZ"], 1)
                vector.memset(runmax[:], 0.0)
                vector.drain()
                # audio: ssq -> 1/ssq  (wait both DMAs: completions unordered)
                for grp in range(2):
                    if grp == 0:
                        vector.wait_ge(sems["ADMA"], 32)
                    vector.tensor_mul(sqtmp[:], a_nat[grp][:], a_nat[grp][:])
                    vector.drain()
                    vector.tensor_reduce(
                        out=rs_a[grp][:], in_=sqtmp[:], axis=X, op=Alu.add
                    )
                    vector.drain()
                    vector.reciprocal(rs_a[grp][:], rs_a[grp][:]).then_inc(
                        sems["DVA"], 1
                    )
                # a_bf = a_nat * rstd  (rstd = sqrt(1/ssq) via ACT)
                for grp in range(2):
                    vector.wait_ge(sems["ACTA"], grp + 1)
                    vector.tensor_scalar_mul(
                        a_bf[grp][:], a_nat[grp][:], rs_a[grp][:]
                    ).then_inc(sems["DVB"], 1)
                # aT collection from transpose psums
                for t in range(16):
                    grp, ch = divmod(t, DCH)
                    vector.wait_ge(sems["PET"], t + 1)
                    vector.tensor_copy(
                        out=aT[:, ch, grp * AGRP : (grp + 1) * AGRP],
                        in_=apsum[t % 2][:],
                    ).then_inc(sems["ATR"], 1)

                vector.wait_ge(sems["ID"], 1)

                def stt_slot(x):
                    """scale+max, min capture, and (at clip end) the
                    partition-max tree + per-clip segment sums for x."""
                    sl = ps_s[x % 3][:]
                    sv = rs[:, x % 4 : x % 4 + 1]
                    vector.wait_ge(sems["SQRT"], x + 1)
                    vector.drain()
                    if x % VBLK == 0:
                        vector.tensor_scalar_mul(runmax[:, 0:ATOK], sl, sv)
                    else:
                        vector.scalar_tensor_tensor(
                            out=runmax[:, 0:ATOK], in0=sl, scalar=sv,
                            in1=runmax[:, 0:ATOK], op0=Alu.mult, op1=Alu.max,
                        )
                    vector.tensor_scalar_min(
                        neg[:, x % 4, :], sl, 0.0
                    ).then_inc(sems["PSFREE"], 1)
                    if x % VBLK == VBLK - 1:
                        # max over the 128 v-lanes: per 32-partition band,
                        # 32x32 block-transpose then free-axis max; combine
                        # bands elementwise (DVE can't mix base partitions).
                        jj = x // VBLK
                        mslice = maxtok[:, jj * 7 : (jj + 1) * 7]
                        for b in range(4):
                            vector.drain()
                            vector.transpose(
                                trout[:], runmax[32 * b : 32 * (b + 1), :]
                            )
                            vector.drain()
                            if b == 0:
                                red = vector.tensor_reduce(
                                    out=mslice,
                                    in_=trout[:].rearrange(
                                        "p (b v) -> p b v", v=32
                                    ),
                                    axis=X, op=Alu.max,
                                )
                            else:
                                vector.tensor_reduce(
                                    out=bandtmp[:],
                                    in_=trout[:].rearrange(
                                        "p (b v) -> p b v", v=32
                                    ),
                                    axis=X, op=Alu.max,
                                )
                                vector.drain()
                                red = vector.tensor_max(
                                    mslice, mslice, bandtmp[:]
                                )
                        red.then_inc(sems["DVEDONE"], 1)

                for g in range(G):
                    q = g % 4
                    vector.wait_ge(sems["PEVB"], g + 1)
                    vector.tensor_mul(
                        dtmp[g % 2][:], ps_n[g % 3][:], ident[:]
                    )
                    vector.drain()
                    vector.tensor_reduce(
                        out=ssq[:, q : q + 1], in_=dtmp[g % 2][:], axis=X,
                        op=Alu.add,
                    )
                    vector.drain()
                    if g >= 4:
                        # protects rs slot (ACT square(g-4) read) and
                        # neg slot (ACT square(g-4) read) before reuse
                        vector.wait_ge(sems["ACTSQ"], g - 3)
                        vector.wait_ge(sems["SQRT"], g - 3)
                    vector.reciprocal(
                        rs[:, q : q + 1], ssq[:, q : q + 1]
                    ).then_inc(sems["RECIP"], 1)
                    if g >= 1:
                        stt_slot(g - 1)
                stt_slot(G - 1)

                # nonneg final reduction (128 partials; host sums them)
                vector.wait_ge(sems["ACTSQ"], G)
                vector.tensor_reduce(
                    out=nn_col[:], in_=nn_buf[:], axis=X, op=Alu.add
                ).then_inc(sems["DVEDONE"], 1)

            @block.scalar
            def _(scalar):
                for grp in range(2):
                    scalar.wait_ge(sems["DVA"], grp + 1)
                    scalar.activation(
                        out=rs_a[grp][:], in_=rs_a[grp][:], func=Act.Sqrt
                    ).then_inc(sems["ACTA"], 1)

                def square_slot(x):
                    scalar.wait_ge(sems["PSFREE"], x + 1)
                    scalar.drain()
                    scalar.activation(
                        out=sqjunk[:], in_=neg[:, x % 4, :], func=Act.Square,
                        scale=rs[:, x % 4 : x % 4 + 1],
                        accum_out=nn_buf[:, x : x + 1],
                    ).then_inc(sems["ACTSQ"], 1)

                for y in range(G):
                    scalar.wait_ge(sems["RECIP"], y + 1)
                    scalar.activation(
                        out=rs[:, y % 4 : y % 4 + 1],
                        in_=rs[:, y % 4 : y % 4 + 1], func=Act.Sqrt,
                    ).then_inc(sems["SQRT"], 1)
                    if y >= 1:
                        square_slot(y - 1)
                square_slot(G - 1)

    return nc


def _get_nc():
    global _CACHED_NC
    if _CACHED_NC is None:
        _CACHED_NC = _build_bass()
    return _CACHED_NC


def kernel(audio_feats, visual_feats, temperature):
    global LAST_EXEC_NS, LAST_TRACE_PATH
    from concourse.bass_utils import run_bass_kernel_spmd

    audio = np.ascontiguousarray(np.asarray(audio_feats, dtype=np.float32))
    visual = np.ascontiguousarray(np.asarray(visual_feats, dtype=np.float32))
    t = float(np.asarray(temperature, dtype=np.float32).reshape(()))

    # (B, Nv, D) -> (B, D, Nv): put the contraction dim first per clip so
    # the device can DMA d-major tiles straight onto SBUF partitions.
    vt = np.ascontiguousarray(visual.transpose(0, 2, 1))

    nc = _get_nc()
    in_maps = [
        {
            "audio": np.ascontiguousarray(
                audio[c * CPC : (c + 1) * CPC].reshape(ATOK, D)
            ),
            "vt": vt,
        }
        for c in range(N_CORES)
    ]
    res = run_bass_kernel_spmd(nc, in_maps, list(range(N_CORES)), trace=TRACE)
    LAST_EXEC_NS = res.exec_time_ns
    if res.instructions_and_trace is not None:
        LAST_TRACE_PATH = res.instructions_and_trace[1]
    results = res.results

    cs = np.zeros((B, B), np.float64)
    nn_sum = 0.0
    for c in range(N_CORES):
        mt = np.asarray(results[c]["clip_rows"], np.float64)  # (32, 7*B)
        nn_sum += float(np.asarray(results[c]["nonneg"], np.float64).sum())
        for j in range(B):
            # token a = 32b + i lives at mt[i, j*7 + b]; a >= 200 is pad
            token_max = mt[:, j * 7 : (j + 1) * 7].T.reshape(224)[:ATOK]
            for il in range(CPC):
                cs[c * CPC + il, j] = token_max[il * NA : (il + 1) * NA].sum()

    cs /= NA * t                  # mean over audio tokens + temperature
    scaled = cs / np.sqrt(float(B))

    def lsm(m):
        s = m - m.max(axis=1, keepdims=True)
        return s - np.log(np.exp(s).sum(axis=1, keepdims=True))

    diag = np.arange(B)
    lp_a2v = lsm(scaled)
    lp_v2a = lsm(scaled.T)
    contrastive = float(-(lp_a2v[diag, diag] + lp_v2a[diag, diag]).mean() / 2.0)

    l_nonneg = (nn_sum / (t * t)) / float(B * B * NA * NV)
    l_cal = max(-np.log(t), 0.0) ** 2
    return np.float32(contrastive + 0.01 * l_nonneg + 0.1 * l_cal)
